# revision 20
# baseline (speedup 1.0000x reference)
"""Trainium2 Bass kernel for nn_EnvironmentEmbedder.

Sharding: pure data parallel. Core i processes batch slice [128*i : 128*(i+1)],
with batch elements mapped to SBUF partitions ([128, free] tiles everywhere).

Transport precision: the kernel is HBM-bound, so everything that tolerates
rounding is moved in bf16 and upcast on the host afterwards:
  - output tensor is computed/stored as bf16 (one final rounding, rel err
    <= 2^-9, well inside the 2e-2 gate) and upcast to f32 on the host;
  - previous_visitations / all_previous_targets / previous_target and the
    non-obs [B,H,W] singles stream in as bf16 (downstream ops are products
    and positive sums only — no cancellation — with fp32 accumulation in
    reductions);
  - embedded_static / embedded_dynamic stay f32: rounding the addends
    before the add would blow up element-wise rel err where static ~=
    -dynamic, so the add runs on-device in f32 and only the final product
    rounds to bf16. observability_in_memory also stays f32 since it
    multiplies every channel.
Per-core traffic: 140.8 MB (f32 everywhere) -> 110.9 MB.

Engine split (DMA is the critical path; keep both HWDGE queues unblocked):
  - SP (sync) queue: loads ONLY — it never waits on compute, so input
    streaming cannot be head-of-line-blocked by a store whose producer
    hasn't finished (that stalled ~20% of DMA time in an earlier rev).
  - Activation (scalar) queue: all stores (the engine blocks on the
    producing op's semaphore, which is harmless there) + the 6 tiny
    compass-channel broadcasts + the 18 shuffle first-ops.
  - Vector: env adds/muls, premultiplies, shuffle accumulates, reductions.
    (GpSimd is useless here: elementwise is ~2x slower there and it shares
    its SBUF port with Vector; a gpsimd tensor_add also wedged the device.)

Per-core compute layout (output = [128, 161*625] bf16, channel-major):
  ch   0..127  (static_c + dynamic_c) * obs      streamed in 8-channel chunks
  ch 128       obstacle * obs
  ch 129       observability_current * obs
  ch 130       obs * obs
  ch 131..136  shuffle(prev_visitations)_j * 0.5 * obs
  ch 137       sum_k(vis_k) * obs
  ch 138       leader * obs
  ch 139       follower * obs
  ch 140..145  shuffle(all_prev_targets)_j * 0.5 * obs
  ch 146..151  shuffle(previous_target)_j * obs
  ch 152       0.5 * sum_k(atgt_k) * obs
  ch 153       sum_k(ptgt_k) * obs
  ch 154       1.0
  ch 155..160  one_hot(rot)
where obs := observability_in_memory.

The egocentric shuffle out_j = x_{(j - rot) % 6} is computed with per-partition
one-hot masks R_r = (rot == r):  out_j = sum_r R_r * x_{(j-r)%6}.  The obs
multiply is folded in by premultiplying the 6 source channels by obs once, and
the 0.5 scaling is folded into the masks.
"""

import sys

sys.path.insert(0, "/opt/trn_rl_repo")

from contextlib import ExitStack

import ml_dtypes
import numpy as np

import concourse.bass as bass
import concourse.tile as tile
from concourse import bacc, mybir
from concourse.bass_utils import run_bass_kernel_spmd

F32 = mybir.dt.float32
BF16 = mybir.dt.bfloat16
I32 = mybir.dt.int32
ALU = mybir.AluOpType

B = 1024
N_CORES = 8
BS = B // N_CORES  # 128 batch elements per core = SBUF partitions
EMB = 128
HW = 625  # 25*25
NROT = 6
NCH = EMB + 33  # 161 output channels

ENV_CHUNK = 8  # env channels per streamed tile
PACK32_LAYOUT = [("obs", HW), ("rot", 1)]
PACK32_W = sum(w for _, w in PACK32_LAYOUT)  # 626
PACK16_LAYOUT = [("obstacle", HW), ("ocur", HW), ("leader", HW),
                 ("follower", HW), ("vis", NROT * HW), ("atgt", NROT * HW),
                 ("ptgt", NROT * HW)]
PACK16_W = sum(w for _, w in PACK16_LAYOUT)  # 13750
STAGE_CHUNKS = [(128, 6), (134, 6), (140, 6), (146, 6), (152, 6),
                (158, 3)]  # (start_ch, n_ch)


def build_body(nc, tc, ctx, t_in, t_out):
    pool = ctx.enter_context(tc.tile_pool(name="resident", bufs=1))
    stage_pool = ctx.enter_context(tc.tile_pool(name="stage", bufs=3))
    env_s_pool = ctx.enter_context(tc.tile_pool(name="env_s", bufs=4))
    env_d_pool = ctx.enter_context(tc.tile_pool(name="env_d", bufs=2))
    env_o_pool = ctx.enter_context(tc.tile_pool(name="env_o", bufs=2))

    # ---- resident loads ----
    # pack32 (obs + rot, 0.3 MB) lands first so the mask/obs_rep setup is
    # done by the time env chunk 0 arrives; pack16 is deferred until after
    # the chunk-0 loads are queued (see env stream below).
    pack32_t = pool.tile([BS, PACK32_W], F32, tag="pack32")
    nc.sync.dma_start(pack32_t[:], t_in["pack32"][:])
    pack16_t = pool.tile([BS, PACK16_W], BF16, tag="pack16")
    cols = {}
    off = 0
    for name, wdt in PACK32_LAYOUT:
        cols[name] = pack32_t[:, off:off + wdt]
        off += wdt
    off = 0
    for name, wdt in PACK16_LAYOUT:
        cols[name] = pack16_t[:, off:off + wdt]
        off += wdt
    obs_t = cols["obs"]
    obst_t = cols["obstacle"]
    ocur_t = cols["ocur"]
    lead_t = cols["leader"]
    foll_t = cols["follower"]
    vis_t = cols["vis"]
    atgt_t = cols["atgt"]
    ptgt_t = cols["ptgt"]
    rot_t = cols["rot"].bitcast(I32)

    # ---- constants: masks, replicated obs ----
    # masks in bf16 (0/1/0.5 are exact): with every STT operand 16-bit the
    # DVE runs the 90 shuffle accumulate ops at 2x. f32 copies kept for the
    # compass activation bias.
    Rf = []   # Rf[r]  = (rot == r)           [128, 1] f32 (Activation scale/bias)
    Rhf = []  # Rhf[r] = 0.5 * (rot == r)     [128, 1] f32 (Activation scale)
    R = []    # R[r]   = (rot == r)           [128, 1] bf16 (DVE STT scalar)
    Rh = []   # Rh[r]  = 0.5 * (rot == r)     [128, 1] bf16
    for r in range(NROT):
        rf = pool.tile([BS, 1], F32, tag=f"Rf{r}")
        nc.vector.tensor_scalar(rf[:], rot_t, r, None, op0=ALU.is_equal)
        Rf.append(rf)
        rhf = pool.tile([BS, 1], F32, tag=f"Rhf{r}")
        nc.vector.tensor_scalar_mul(rhf[:], rf[:], 0.5)
        Rhf.append(rhf)
        rt = pool.tile([BS, 1], BF16, tag=f"R{r}")
        nc.vector.tensor_copy(rt[:], rf[:])
        R.append(rt)
        rh = pool.tile([BS, 1], BF16, tag=f"Rh{r}")
        nc.vector.tensor_scalar_mul(rh[:], rf[:], 0.5)
        Rh.append(rh)

    # obs replicated to NROT copies (premultiplies use all 6, the env mul
    # uses [0:3750] + [0:1250] in two slices per 8-channel chunk)
    obs_rep = pool.tile([BS, NROT * HW], F32, tag="obs_rep")
    for k in range(NROT):
        nc.vector.tensor_copy(obs_rep[:, k * HW:(k + 1) * HW], obs_t)

    def emit_premults():
        # premultiply the 6-channel tensors by obs (in place, bf16)
        for xt in (vis_t, atgt_t, ptgt_t):
            nc.vector.tensor_mul(xt, xt, obs_rep[:])

    def emit_shuffle(slot, xp, masks, masks_f32, j):
        # slot = sum_r masks[r] * xp[:, ((j - r) % 6)]
        # first op on the (otherwise idle) Activation engine, which needs
        # an f32 scale AP; the DVE accumulate ops take the bf16 masks
        nc.scalar.mul(slot, xp[:, j * HW:(j + 1) * HW], masks_f32[0][:])
        for r in range(1, NROT):
            k = (j - r) % NROT
            nc.vector.scalar_tensor_tensor(
                slot, xp[:, k * HW:(k + 1) * HW], masks[r][:], slot,
                op0=ALU.mult, op1=ALU.add)

    chsum_scratch = pool.tile([BS, HW], F32, tag="chsum_scratch")

    def emit_chsum(slot, xp, scale=None):
        # fp32 accumulate, then one rounding into the bf16 slot
        nc.vector.tensor_reduce(
            chsum_scratch[:], xp.rearrange("p (c x) -> p x c", c=NROT),
            axis=mybir.AxisListType.X, op=ALU.add)
        if scale is None:
            nc.vector.tensor_copy(slot, chsum_scratch[:])
        else:
            nc.vector.tensor_scalar_mul(slot, chsum_scratch[:], scale)

    def emit_channel(ch, slot):
        if ch == 128:
            nc.vector.tensor_mul(slot, obst_t, obs_t)
        elif ch == 129:
            nc.vector.tensor_mul(slot, ocur_t, obs_t)
        elif ch == 130:
            nc.vector.tensor_mul(slot, obs_t, obs_t)
        elif 131 <= ch <= 136:
            emit_shuffle(slot, vis_t, Rh, Rhf, ch - 131)
        elif ch == 137:
            emit_chsum(slot, vis_t)
        elif ch == 138:
            nc.vector.tensor_mul(slot, lead_t, obs_t)
        elif ch == 139:
            nc.vector.tensor_mul(slot, foll_t, obs_t)
        elif 140 <= ch <= 145:
            emit_shuffle(slot, atgt_t, Rh, Rhf, ch - 140)
        elif 146 <= ch <= 151:
            emit_shuffle(slot, ptgt_t, R, Rf, ch - 146)
        elif ch == 152:
            emit_chsum(slot, atgt_t, scale=0.5)
        elif ch == 153:
            emit_chsum(slot, ptgt_t)
        elif ch == 154:
            nc.vector.memset(slot, 1.0)
        else:  # 155..160: compass one-hot = Identity(0*obs + R[r])
            nc.scalar.activation(
                slot, obs_t, mybir.ActivationFunctionType.Identity,
                bias=Rf[ch - 155][:], scale=0.0)

    # ---- env stream interleaved with the small channels ----
    ch_queue = []
    for ck, (start_ch, n_ch) in enumerate(STAGE_CHUNKS):
        for i in range(n_ch):
            ch_queue.append((ck, start_ch, n_ch, i))
    stage_tiles = {}
    # A stage store is deferred until the NEXT stage chunk finishes: by then
    # its producing DVE ops are long done, so the store's semaphore wait is
    # pre-resolved and never head-of-line-blocks the env stores behind it in
    # the Activation queue (that blocking cascaded into env_o WAR stalls of
    # the Vector muls and then into load stalls).
    pending_stage = []  # [(out_cols, tile)]

    def emit_small(budget):
        while budget > 0 and ch_queue:
            ck, start_ch, n_ch, i = ch_queue.pop(0)
            if ck not in stage_tiles:
                stage_tiles[ck] = stage_pool.tile(
                    [BS, n_ch * HW], BF16, tag="stage", name=f"stage{ck}")
            emit_channel(start_ch + i, stage_tiles[ck][:, i * HW:(i + 1) * HW])
            if i == n_ch - 1:
                while pending_stage:
                    out_cols, tile_ = pending_stage.pop(0)
                    nc.scalar.dma_start(t_out[:, out_cols], tile_[:])
                pending_stage.append(
                    (slice(start_ch * HW, (start_ch + n_ch) * HW),
                     stage_tiles[ck]))
            budget -= 1

    w = ENV_CHUNK * HW
    rep_w = NROT * HW  # obs_rep width (3750) < chunk width (5000)
    env_total = EMB // ENV_CHUNK
    pending_env = []  # [(out_cols, tile)] env stores deferred one chunk
    for c in range(env_total):
        cols_ = slice(c * w, (c + 1) * w)
        if c == 2:
            # resident bf16 pack: queued after the chunk-0/1 loads so the
            # env pipeline starts immediately; it feeds the premultiplies,
            # needed only by the small channels, which start at c >= 2
            nc.sync.dma_start(pack16_t[:], t_in["pack16"][:])
            emit_premults()
        s_tile = env_s_pool.tile([BS, w], F32, tag="env_s")
        nc.sync.dma_start(s_tile[:], t_in["embedded_static"][:, cols_])
        d_tile = env_d_pool.tile([BS, w], F32, tag="env_d")
        nc.sync.dma_start(d_tile[:], t_in["embedded_dynamic"][:, cols_])
        nc.vector.tensor_add(s_tile[:], s_tile[:], d_tile[:])
        o_tile = env_o_pool.tile([BS, w], BF16, tag="env_o")
        nc.vector.tensor_mul(o_tile[:, :rep_w], s_tile[:, :rep_w],
                             obs_rep[:])
        nc.vector.tensor_mul(o_tile[:, rep_w:], s_tile[:, rep_w:],
                             obs_rep[:, :w - rep_w])
        # env store also deferred one chunk: by issue time its mul has long
        # finished, so the Activation queue never blocks on it
        while pending_env:
            out_cols, tile_ = pending_env.pop(0)
            nc.scalar.dma_start(t_out[:, out_cols], tile_[:])
        pending_env.append((cols_, o_tile))
        if c >= 2:
            emit_small(4)
    emit_small(len(ch_queue))
    for out_cols, tile_ in pending_env:
        nc.scalar.dma_start(t_out[:, out_cols], tile_[:])
    pending_env.clear()
    for out_cols, tile_ in pending_stage:
        nc.scalar.dma_start(t_out[:, out_cols], tile_[:])
    pending_stage.clear()


def build_nc():
    nc = bacc.Bacc("TRN2", target_bir_lowering=False, debug=False)
    t_in = {
        "embedded_static": nc.dram_tensor(
            "embedded_static", [BS, EMB * HW], F32, kind="ExternalInput"),
        "embedded_dynamic": nc.dram_tensor(
            "embedded_dynamic", [BS, EMB * HW], F32, kind="ExternalInput"),
        "pack32": nc.dram_tensor(
            "pack32", [BS, PACK32_W], F32, kind="ExternalInput"),
        "pack16": nc.dram_tensor(
            "pack16", [BS, PACK16_W], BF16, kind="ExternalInput"),
    }
    t_out = nc.dram_tensor("out", [BS, NCH * HW], BF16, kind="ExternalOutput")
    with tile.TileContext(nc) as tc, ExitStack() as ctx:
        build_body(nc, tc, ctx, t_in, t_out)
    nc.compile()
    return nc


def make_in_maps(inputs):
    arrs = {k: np.asarray(v) for k, v in inputs.items()}
    src32 = {
        "obs": arrs["observability_in_memory"].reshape(B, HW),
        "rot": arrs["rotations"].reshape(B, 1).astype(np.int32).view(np.float32),
    }
    src16 = {
        "obstacle": arrs["obstacle_mask"].reshape(B, HW),
        "ocur": arrs["observability_current"].reshape(B, HW),
        "leader": arrs["leader_location"].reshape(B, HW),
        "follower": arrs["follower_location"].reshape(B, HW),
        "vis": arrs["previous_visitations"].reshape(B, NROT * HW),
        "atgt": arrs["all_previous_targets"].reshape(B, NROT * HW),
        "ptgt": arrs["previous_target"].reshape(B, NROT * HW),
    }
    flat = {
        "embedded_static": np.ascontiguousarray(
            arrs["embedded_static"].reshape(B, EMB * HW)),
        "embedded_dynamic": np.ascontiguousarray(
            arrs["embedded_dynamic"].reshape(B, EMB * HW)),
        "pack32": np.concatenate(
            [src32[name] for name, _ in PACK32_LAYOUT], axis=1),
        "pack16": np.concatenate(
            [src16[name] for name, _ in PACK16_LAYOUT],
            axis=1).astype(ml_dtypes.bfloat16),
    }
    return [
        {k: v[i * BS:(i + 1) * BS] for k, v in flat.items()}
        for i in range(N_CORES)
    ]


def kernel(**inputs) -> np.ndarray:
    nc = build_nc()
    in_maps = make_in_maps(inputs)
    res = run_bass_kernel_spmd(nc, in_maps, list(range(N_CORES)))
    return np.concatenate(
        [r["out"].astype(np.float32).reshape(BS, NCH, 25, 25)
         for r in res.results], axis=0)


if __name__ == "__main__":
    rng = np.random.default_rng(0)
    demo = {
        "embedded_static": rng.standard_normal((B, EMB, 25, 25), np.float32),
        "embedded_dynamic": rng.standard_normal((B, EMB, 25, 25), np.float32),
        "obstacle_mask": rng.random((B, 25, 25), dtype=np.float32),
        "observability_current": rng.random((B, 25, 25), dtype=np.float32),
        "observability_in_memory": rng.random((B, 25, 25), dtype=np.float32),
        "previous_visitations": rng.random((B, NROT, 25, 25), dtype=np.float32),
        "all_previous_targets": rng.random((B, NROT, 25, 25), dtype=np.float32),
        "previous_target": rng.random((B, NROT, 25, 25), dtype=np.float32),
        "leader_location": rng.random((B, 25, 25), dtype=np.float32),
        "follower_location": rng.random((B, 25, 25), dtype=np.float32),
        "rotations": rng.integers(0, NROT, (B,), dtype=np.int32),
    }
    out = kernel(**demo)
    print("out", out.shape, out.dtype)


# revision 22
# speedup vs baseline: 1.1763x; 1.1763x over previous
"""Trainium2 Bass kernel for nn_EnvironmentEmbedder.

Sharding: pure data parallel. Core i processes batch slice [128*i : 128*(i+1)],
with batch elements mapped to SBUF partitions ([128, free] tiles everywhere).

Transport precision: the kernel is HBM-bound, so everything that tolerates
rounding is moved in bf16 and upcast on the host afterwards:
  - output tensor is computed/stored as bf16 (one final rounding, rel err
    <= 2^-9, well inside the 2e-2 gate) and upcast to f32 on the host;
  - previous_visitations / all_previous_targets / previous_target and the
    non-obs [B,H,W] singles stream in as bf16 (downstream ops are products
    and positive sums only — no cancellation — with fp32 accumulation in
    reductions);
  - embedded_static / embedded_dynamic stay f32: rounding the addends
    before the add would blow up element-wise rel err where static ~=
    -dynamic, so the add runs on-device in f32 and only the final product
    rounds to bf16. observability_in_memory also stays f32 since it
    multiplies every channel.
Per-core traffic: 140.8 MB (f32 everywhere) -> 110.9 MB.

Engine split (DMA is the critical path; keep both HWDGE queues unblocked):
  - SP (sync) queue: loads ONLY — it never waits on compute, so input
    streaming cannot be head-of-line-blocked by a store whose producer
    hasn't finished (that stalled ~20% of DMA time in an earlier rev).
  - Activation (scalar) queue: all stores (the engine blocks on the
    producing op's semaphore, which is harmless there) + the 6 tiny
    compass-channel broadcasts + the 18 shuffle first-ops.
  - Vector: env adds/muls, premultiplies, shuffle accumulates, reductions.
    (GpSimd is useless here: elementwise is ~2x slower there and it shares
    its SBUF port with Vector; a gpsimd tensor_add also wedged the device.)

Per-core compute layout (output = [128, 161*625] bf16, channel-major):
  ch   0..127  (static_c + dynamic_c) * obs      streamed in 8-channel chunks
  ch 128       obstacle * obs
  ch 129       observability_current * obs
  ch 130       obs * obs
  ch 131..136  shuffle(prev_visitations)_j * 0.5 * obs
  ch 137       sum_k(vis_k) * obs
  ch 138       leader * obs
  ch 139       follower * obs
  ch 140..145  shuffle(all_prev_targets)_j * 0.5 * obs
  ch 146..151  shuffle(previous_target)_j * obs
  ch 152       0.5 * sum_k(atgt_k) * obs
  ch 153       sum_k(ptgt_k) * obs
  ch 154       1.0
  ch 155..160  one_hot(rot)
where obs := observability_in_memory.

The egocentric shuffle out_j = x_{(j - rot) % 6} is computed with per-partition
one-hot masks R_r = (rot == r):  out_j = sum_r R_r * x_{(j-r)%6}.  The obs
multiply is folded in by premultiplying the 6 source channels by obs once, and
the 0.5 scaling is folded into the masks.
"""

import sys

sys.path.insert(0, "/opt/trn_rl_repo")

from contextlib import ExitStack

import ml_dtypes
import numpy as np

import concourse.tile as tile
from concourse import bacc, mybir
from concourse.bass_utils import run_bass_kernel_spmd

F32 = mybir.dt.float32
BF16 = mybir.dt.bfloat16
I32 = mybir.dt.int32
ALU = mybir.AluOpType

B = 1024
N_CORES = 8
BS = B // N_CORES  # 128 batch elements per core = SBUF partitions
EMB = 128
HW = 625  # 25*25
NROT = 6
NCH = EMB + 33  # 161 output channels

ENV_CHUNK = 8  # env channels per streamed tile
PACK32_LAYOUT = [("obs", HW), ("rot", 1)]
PACK32_W = sum(w for _, w in PACK32_LAYOUT)  # 626
PACK16_LAYOUT = [("obstacle", HW), ("ocur", HW), ("leader", HW),
                 ("follower", HW), ("vis", NROT * HW), ("atgt", NROT * HW),
                 ("ptgt", NROT * HW)]
PACK16_W = sum(w for _, w in PACK16_LAYOUT)  # 13750
STAGE_CHUNKS = [(128, 6), (134, 6), (140, 6), (146, 6), (152, 6),
                (158, 3)]  # (start_ch, n_ch)


def build_body(nc, tc, ctx, t_in, t_out):
    pool = ctx.enter_context(tc.tile_pool(name="resident", bufs=1))
    stage_pool = ctx.enter_context(tc.tile_pool(name="stage", bufs=3))
    env_s_pool = ctx.enter_context(tc.tile_pool(name="env_s", bufs=4))
    env_d_pool = ctx.enter_context(tc.tile_pool(name="env_d", bufs=2))
    env_o_pool = ctx.enter_context(tc.tile_pool(name="env_o", bufs=2))

    # ---- resident loads ----
    # pack32 (obs + rot, 0.3 MB) lands first so the mask/obs_rep setup is
    # done by the time env chunk 0 arrives; pack16 is deferred until after
    # the chunk-0 loads are queued (see env stream below).
    pack32_t = pool.tile([BS, PACK32_W], F32, tag="pack32")
    nc.sync.dma_start(pack32_t[:], t_in["pack32"][:])
    pack16_t = pool.tile([BS, PACK16_W], BF16, tag="pack16")
    cols = {}
    off = 0
    for name, wdt in PACK32_LAYOUT:
        cols[name] = pack32_t[:, off:off + wdt]
        off += wdt
    off = 0
    for name, wdt in PACK16_LAYOUT:
        cols[name] = pack16_t[:, off:off + wdt]
        off += wdt
    obs_t = cols["obs"]
    obst_t = cols["obstacle"]
    ocur_t = cols["ocur"]
    lead_t = cols["leader"]
    foll_t = cols["follower"]
    vis_t = cols["vis"]
    atgt_t = cols["atgt"]
    ptgt_t = cols["ptgt"]
    rot_t = cols["rot"].bitcast(I32)

    # ---- constants: masks, replicated obs ----
    # masks kept in bf16 (0/1/0.5 are exact) for the DVE STT ops, plus f32
    # copies for the Activation engine, whose scale AP must be FP32.
    # (Measured: all-16-bit operands do NOT speed up the 3-AP STT ops; the
    # bf16 masks are kept because they cost nothing.)
    Rf = []   # Rf[r]  = (rot == r)           [128, 1] f32 (Activation scale/bias)
    Rhf = []  # Rhf[r] = 0.5 * (rot == r)     [128, 1] f32 (Activation scale)
    R = []    # R[r]   = (rot == r)           [128, 1] bf16 (DVE STT scalar)
    Rh = []   # Rh[r]  = 0.5 * (rot == r)     [128, 1] bf16
    for r in range(NROT):
        rf = pool.tile([BS, 1], F32, tag=f"Rf{r}")
        nc.vector.tensor_scalar(rf[:], rot_t, r, None, op0=ALU.is_equal)
        Rf.append(rf)
        rhf = pool.tile([BS, 1], F32, tag=f"Rhf{r}")
        nc.vector.tensor_scalar_mul(rhf[:], rf[:], 0.5)
        Rhf.append(rhf)
        rt = pool.tile([BS, 1], BF16, tag=f"R{r}")
        nc.vector.tensor_copy(rt[:], rf[:])
        R.append(rt)
        rh = pool.tile([BS, 1], BF16, tag=f"Rh{r}")
        nc.vector.tensor_scalar_mul(rh[:], rf[:], 0.5)
        Rh.append(rh)

    # obs replicated to NROT copies (premultiplies use all 6, the env mul
    # uses [0:3750] + [0:1250] in two slices per 8-channel chunk)
    obs_rep = pool.tile([BS, NROT * HW], F32, tag="obs_rep")
    for k in range(NROT):
        nc.vector.tensor_copy(obs_rep[:, k * HW:(k + 1) * HW], obs_t)

    def emit_premults():
        # premultiply the 6-channel tensors by obs (in place, bf16)
        for xt in (vis_t, atgt_t, ptgt_t):
            nc.vector.tensor_mul(xt, xt, obs_rep[:])

    def emit_shuffle(slot, xp, masks, masks_f32, j):
        # slot = sum_r masks[r] * xp[:, ((j - r) % 6)]
        # first op on the (otherwise idle) Activation engine, which needs
        # an f32 scale AP; the DVE accumulate ops take the bf16 masks
        nc.scalar.mul(slot, xp[:, j * HW:(j + 1) * HW], masks_f32[0][:])
        for r in range(1, NROT):
            k = (j - r) % NROT
            nc.vector.scalar_tensor_tensor(
                slot, xp[:, k * HW:(k + 1) * HW], masks[r][:], slot,
                op0=ALU.mult, op1=ALU.add)

    chsum_scratch = pool.tile([BS, HW], F32, tag="chsum_scratch")

    def emit_chsum(slot, xp, scale=None):
        # fp32 accumulate, then one rounding into the bf16 slot
        nc.vector.tensor_reduce(
            chsum_scratch[:], xp.rearrange("p (c x) -> p x c", c=NROT),
            axis=mybir.AxisListType.X, op=ALU.add)
        if scale is None:
            nc.vector.tensor_copy(slot, chsum_scratch[:])
        else:
            nc.vector.tensor_scalar_mul(slot, chsum_scratch[:], scale)

    def emit_channel(ch, slot):
        if ch == 128:
            nc.vector.tensor_mul(slot, obst_t, obs_t)
        elif ch == 129:
            nc.vector.tensor_mul(slot, ocur_t, obs_t)
        elif ch == 130:
            nc.vector.tensor_mul(slot, obs_t, obs_t)
        elif 131 <= ch <= 136:
            emit_shuffle(slot, vis_t, Rh, Rhf, ch - 131)
        elif ch == 137:
            emit_chsum(slot, vis_t)
        elif ch == 138:
            nc.vector.tensor_mul(slot, lead_t, obs_t)
        elif ch == 139:
            nc.vector.tensor_mul(slot, foll_t, obs_t)
        elif 140 <= ch <= 145:
            emit_shuffle(slot, atgt_t, Rh, Rhf, ch - 140)
        elif 146 <= ch <= 151:
            emit_shuffle(slot, ptgt_t, R, Rf, ch - 146)
        elif ch == 152:
            emit_chsum(slot, atgt_t, scale=0.5)
        elif ch == 153:
            emit_chsum(slot, ptgt_t)
        elif ch == 154:
            nc.vector.memset(slot, 1.0)
        else:  # 155..160: compass one-hot = Identity(0*obs + R[r])
            nc.scalar.activation(
                slot, obs_t, mybir.ActivationFunctionType.Identity,
                bias=Rf[ch - 155][:], scale=0.0)

    # ---- env stream interleaved with the small channels ----
    ch_queue = []
    for ck, (start_ch, n_ch) in enumerate(STAGE_CHUNKS):
        for i in range(n_ch):
            ch_queue.append((ck, start_ch, n_ch, i))
    stage_tiles = {}
    # A stage store is deferred until the NEXT stage chunk finishes: by then
    # its producing DVE ops are long done, so the store's semaphore wait is
    # pre-resolved and never head-of-line-blocks the env stores behind it in
    # the Activation queue (that blocking cascaded into env_o WAR stalls of
    # the Vector muls and then into load stalls).
    pending_stage = []  # [(out_cols, tile)]

    def emit_small(budget):
        while budget > 0 and ch_queue:
            ck, start_ch, n_ch, i = ch_queue.pop(0)
            if ck not in stage_tiles:
                stage_tiles[ck] = stage_pool.tile(
                    [BS, n_ch * HW], BF16, tag="stage", name=f"stage{ck}")
            emit_channel(start_ch + i, stage_tiles[ck][:, i * HW:(i + 1) * HW])
            if i == n_ch - 1:
                while pending_stage:
                    out_cols, tile_ = pending_stage.pop(0)
                    nc.scalar.dma_start(t_out[:, out_cols], tile_[:])
                pending_stage.append(
                    (slice(start_ch * HW, (start_ch + n_ch) * HW),
                     stage_tiles[ck]))
            budget -= 1

    w = ENV_CHUNK * HW
    rep_w = NROT * HW  # obs_rep width (3750) < chunk width (5000)
    env_total = EMB // ENV_CHUNK
    for c in range(env_total):
        cols_ = slice(c * w, (c + 1) * w)
        if c == 2:
            # resident bf16 pack: queued after the chunk-0/1 loads so the
            # env pipeline starts immediately; it feeds the premultiplies,
            # needed only by the small channels, which start at c >= 2
            nc.sync.dma_start(pack16_t[:], t_in["pack16"][:])
            emit_premults()
        s_tile = env_s_pool.tile([BS, w], F32, tag="env_s")
        nc.sync.dma_start(s_tile[:], t_in["embedded_static"][:, cols_])
        d_tile = env_d_pool.tile([BS, w], F32, tag="env_d")
        nc.sync.dma_start(d_tile[:], t_in["embedded_dynamic"][:, cols_])
        nc.vector.tensor_add(s_tile[:], s_tile[:], d_tile[:])
        o_tile = env_o_pool.tile([BS, w], BF16, tag="env_o")
        nc.vector.tensor_mul(o_tile[:, :rep_w], s_tile[:, :rep_w],
                             obs_rep[:])
        nc.vector.tensor_mul(o_tile[:, rep_w:], s_tile[:, rep_w:],
                             obs_rep[:, :w - rep_w])
        nc.scalar.dma_start(t_out[:, cols_], o_tile[:])
        if c >= 2:
            emit_small(3)
    emit_small(len(ch_queue))
    for out_cols, tile_ in pending_stage:
        nc.scalar.dma_start(t_out[:, out_cols], tile_[:])
    pending_stage.clear()


def build_nc():
    nc = bacc.Bacc("TRN2", target_bir_lowering=False, debug=False)
    t_in = {
        "embedded_static": nc.dram_tensor(
            "embedded_static", [BS, EMB * HW], F32, kind="ExternalInput"),
        "embedded_dynamic": nc.dram_tensor(
            "embedded_dynamic", [BS, EMB * HW], F32, kind="ExternalInput"),
        "pack32": nc.dram_tensor(
            "pack32", [BS, PACK32_W], F32, kind="ExternalInput"),
        "pack16": nc.dram_tensor(
            "pack16", [BS, PACK16_W], BF16, kind="ExternalInput"),
    }
    t_out = nc.dram_tensor("out", [BS, NCH * HW], BF16, kind="ExternalOutput")
    with tile.TileContext(nc) as tc, ExitStack() as ctx:
        build_body(nc, tc, ctx, t_in, t_out)
    nc.compile()
    return nc


def make_in_maps(inputs):
    arrs = {k: np.asarray(v) for k, v in inputs.items()}
    src32 = {
        "obs": arrs["observability_in_memory"].reshape(B, HW),
        "rot": arrs["rotations"].reshape(B, 1).astype(np.int32).view(np.float32),
    }
    src16 = {
        "obstacle": arrs["obstacle_mask"].reshape(B, HW),
        "ocur": arrs["observability_current"].reshape(B, HW),
        "leader": arrs["leader_location"].reshape(B, HW),
        "follower": arrs["follower_location"].reshape(B, HW),
        "vis": arrs["previous_visitations"].reshape(B, NROT * HW),
        "atgt": arrs["all_previous_targets"].reshape(B, NROT * HW),
        "ptgt": arrs["previous_target"].reshape(B, NROT * HW),
    }
    flat = {
        "embedded_static": np.ascontiguousarray(
            arrs["embedded_static"].reshape(B, EMB * HW)),
        "embedded_dynamic": np.ascontiguousarray(
            arrs["embedded_dynamic"].reshape(B, EMB * HW)),
        "pack32": np.concatenate(
            [src32[name] for name, _ in PACK32_LAYOUT], axis=1),
        "pack16": np.concatenate(
            [src16[name] for name, _ in PACK16_LAYOUT],
            axis=1).astype(ml_dtypes.bfloat16),
    }
    return [
        {k: v[i * BS:(i + 1) * BS] for k, v in flat.items()}
        for i in range(N_CORES)
    ]


def kernel(**inputs) -> np.ndarray:
    nc = build_nc()
    in_maps = make_in_maps(inputs)
    res = run_bass_kernel_spmd(nc, in_maps, list(range(N_CORES)))
    return np.concatenate(
        [r["out"].astype(np.float32).reshape(BS, NCH, 25, 25)
         for r in res.results], axis=0)


if __name__ == "__main__":
    rng = np.random.default_rng(0)
    demo = {
        "embedded_static": rng.standard_normal((B, EMB, 25, 25), np.float32),
        "embedded_dynamic": rng.standard_normal((B, EMB, 25, 25), np.float32),
        "obstacle_mask": rng.random((B, 25, 25), dtype=np.float32),
        "observability_current": rng.random((B, 25, 25), dtype=np.float32),
        "observability_in_memory": rng.random((B, 25, 25), dtype=np.float32),
        "previous_visitations": rng.random((B, NROT, 25, 25), dtype=np.float32),
        "all_previous_targets": rng.random((B, NROT, 25, 25), dtype=np.float32),
        "previous_target": rng.random((B, NROT, 25, 25), dtype=np.float32),
        "leader_location": rng.random((B, 25, 25), dtype=np.float32),
        "follower_location": rng.random((B, 25, 25), dtype=np.float32),
        "rotations": rng.integers(0, NROT, (B,), dtype=np.int32),
    }
    out = kernel(**demo)
    print("out", out.shape, out.dtype)
